# revision 6
# baseline (speedup 1.0000x reference)
"""Trainium2 Bass kernel for CellPathwayAttentionAggregator (segment-reduce).

Math: out[b, s] = sum_{i in set s} softmax_s(attn_logits)[i] * G[b, flat_idx[i]]

Device decomposition (per core): dense bf16 matmul
    out[b, s] = sum_g G[b, g] * W[g, s]
where W[g, s] = softmax-normalized weight of member (g, s), scattered on the
host as pure layout prep (softmax is exact fp32 host math folded into W).

Sharding: 8 cores = 2 batch groups (512 rows) x 4 set groups (512 sets).
Each core accumulates a (512 x 512) output block over K=8192 in fp32 PSUM
(4 batch-subtile PSUM banks, N=512 moving operand). The PE streams the
moving operand at 1 bf16 column/cycle -> 256 MMs x 512 cyc = 54.6us floor;
the program is built so everything else (DMA 16.7MB @ ~350GB/s, copies,
output DMA) hides under or hangs minimally off that floor:

  - k-tiles DMA'd in PAIRS (512KB, 4KB/partition) alternating the Sync/ACT
    HWDGE rings; 8 pair-slots in SBUF; pair 0 is split into 4 half-tile
    DMAs (2 per ring) with dedicated sems so the first real MM starts ~1us
    after block start.
  - PE warmup MMs (N=1, ~25ns) bridge the gap until tile 0 lands,
    keeping the HAM clock-gate ramping; warm operand is DVE-memset at block
    entry (the 4 framework const-AP gpsimd memsets are removed from the
    entry block -- they cost ~1.9us of all-engine barrier wait).
  - no tensor.drain(): bank m's PSUM->SBUF copy is gated on the *retire*
    of the k=63 MM of bank m+1 (MM ends are strictly ordered, so one-MM
    margin covers the systolic drain); bank 3 is covered by a trailing
    dummy MM. Copies run split DVE/ACT, then one 512KB output DMA with no
    completion wait (the Block-exit queue flush guarantees delivery).
"""

import sys

if "/opt/trn_rl_repo" not in sys.path:
    sys.path.insert(0, "/opt/trn_rl_repo")

import ml_dtypes
import numpy as np

NUM_SETS = 2048
NUM_GENESETS = 8192
BATCH = 1024
N_CORES = 8
BG, SG = 2, 4  # batch groups x set groups (BG*SG == N_CORES)
B_C = BATCH // BG  # 512 batch rows per core
S_C = NUM_SETS // SG  # 512 sets per core
P = 128
K_TILES = NUM_GENESETS // P  # 64
M_TILES = B_C // P  # 4
PAIRS = K_TILES // 2  # 32
SLOTS = 8  # pair slots in SBUF (16 tiles, 4MB)
FD = B_C + S_C  # 1024 bf16 per partition per k-tile
PFD = 2 * FD  # one pair
WARMUPS = 48

_PROGRAM_CACHE = {}
LAST_RESULTS = None  # BassKernelResults of the most recent run (for profiling)


def _strip_const_ap_memsets(nc):
    """Remove the 4 framework const-AP gpsimd memsets from the entry block.
    They run on GpSimd while every other engine sits at the init barrier
    (~1.9us); this program doesn't use any const AP."""
    try:
        import concourse.mybir as mybir

        entry = nc.main_func.blocks[0]
        drop = [
            inst
            for inst in entry.instructions
            if isinstance(inst, mybir.InstMemset)
            and inst.outs
            and "const-" in getattr(inst.outs[0], "memsetref", "")
        ]
        for inst in drop:
            entry.instructions.remove(inst)
    except Exception:
        pass


def _build_program():
    import concourse.mybir as mybir
    from concourse import bacc
    from contextlib import ExitStack

    f32 = mybir.dt.float32
    bf16 = mybir.dt.bfloat16

    nc = bacc.Bacc("TRN2", target_bir_lowering=False, debug=False)
    _strip_const_ap_memsets(nc)
    gw_d = nc.dram_tensor("gw", [PAIRS, P, PFD], bf16, kind="ExternalInput")
    # out rows are (m, p)-major on the device; host transposes for free
    out_d = nc.dram_tensor("out", [P, M_TILES * S_C], bf16, kind="ExternalOutput")

    with ExitStack() as ctx:
        gw_sb = ctx.enter_context(nc.sbuf_tensor([P, SLOTS, PFD], bf16))
        warm_sb = ctx.enter_context(nc.sbuf_tensor([1, 2], bf16))  # never written
        act_sb = ctx.enter_context(nc.sbuf_tensor([1, 2], f32))
        o_sb = ctx.enter_context(nc.sbuf_tensor([P, M_TILES * S_C], bf16))
        acc_ps = ctx.enter_context(nc.psum_tensor([P, M_TILES, S_C], f32))
        warm_ps = ctx.enter_context(nc.psum_tensor([1, 1], f32))
        s_t0 = ctx.enter_context(nc.semaphore(name="s_t0"))
        s_t1 = ctx.enter_context(nc.semaphore(name="s_t1"))
        s_slot = [
            ctx.enter_context(nc.semaphore(name=f"s_slot{j}")) for j in range(SLOTS)
        ]
        s_mm = ctx.enter_context(nc.semaphore(name="s_mm"))
        s_fin = ctx.enter_context(nc.semaphore(name="s_fin"))
        s_outA = ctx.enter_context(nc.semaphore(name="s_outA"))
        s_done = ctx.enter_context(nc.semaphore(name="s_done"))
        s_warm = ctx.enter_context(nc.semaphore(name="s_warm"))
        block = ctx.enter_context(nc.Block(no_gpsimd_drain=True))

        def ring_body(eng, r):
            # pair 0 is handled separately; even pairs on Sync, odd on ACT
            for p_ in range(2 - r, PAIRS, 2):
                if p_ >= SLOTS:
                    eng.wait_ge(s_mm, p_ - SLOTS + 1)
                eng.dma_start(gw_sb[:, p_ % SLOTS, :], gw_d[p_, :, :]).then_inc(
                    s_slot[p_ % SLOTS], 16
                )

        @block.sync
        def _(sync):
            # tile 0 / tile 1 top halves
            sync.dma_start(gw_sb[0:64, 0, 0:FD], gw_d[0, 0:64, 0:FD]).then_inc(
                s_t0, 16
            )
            sync.dma_start(gw_sb[0:64, 0, FD:PFD], gw_d[0, 0:64, FD:PFD]).then_inc(
                s_t1, 16
            )
            ring_body(sync, 0)  # pairs 2, 4, ...
            sync.wait_ge(s_outA, 4)
            sync.dma_start(out_d[:, :], o_sb[:, :]).then_inc(s_done, 16)
            # no completion wait: Block-exit queue flush delivers the bytes

        @block.scalar
        def _(scalar):
            # tile 0 / tile 1 bottom halves
            scalar.dma_start(gw_sb[64:128, 0, 0:FD], gw_d[0, 64:128, 0:FD]).then_inc(
                s_t0, 16
            )
            scalar.dma_start(
                gw_sb[64:128, 0, FD:PFD], gw_d[0, 64:128, FD:PFD]
            ).then_inc(s_t1, 16)
            ring_body(scalar, 1)  # pairs 1, 3, ...
            # dummy act: hoists the ~1.3us ACT_TABLE_LOAD into DMA-paced time
            scalar.activation(
                act_sb[0:1, 0:1], act_sb[0:1, 1:2], mybir.ActivationFunctionType.Copy
            )
            for m in (2, 3):
                scalar.wait_ge(s_fin, m + 1 + (1 if m == 3 else 0))
                scalar.activation(
                    o_sb[:, m * S_C : (m + 1) * S_C],
                    acc_ps[:, m, :],
                    mybir.ActivationFunctionType.Copy,
                ).then_inc(s_outA, 1)

        @block.vector
        def _(vector):
            vector.memset(warm_sb[:], 1.0).then_inc(s_warm, 1)
            for m in (0, 1):
                vector.wait_ge(s_fin, m + 1)
                vector.tensor_copy(
                    o_sb[:, m * S_C : (m + 1) * S_C], acc_ps[:, m, :]
                ).then_inc(s_outA, 1)

        @block.tensor
        def _(tensor):
            # dependency-free warmups keep the HAM clock-gate ramping
            # while tile 0 streams in (warm_sb memset by DVE at block entry)
            tensor.matmul(
                warm_ps[:], warm_sb[0:1, 0:1], warm_sb[0:1, 1:2],
                start=True, stop=True,
            )._wait_ge(s_warm, 1)
            for _ in range(WARMUPS - 1):
                tensor.matmul(
                    warm_ps[:], warm_sb[0:1, 0:1], warm_sb[0:1, 1:2],
                    start=True, stop=True,
                )
            for k in range(K_TILES):
                p_ = k // 2
                j = p_ % SLOTS
                base = (k % 2) * FD
                tile = gw_sb[:, j, base : base + FD]
                for m in range(M_TILES):
                    mm = tensor.matmul(
                        acc_ps[:, m, :],
                        tile[:, m * P : (m + 1) * P],
                        tile[:, B_C:FD],
                        start=(k == 0),
                        stop=(k == K_TILES - 1),
                    )
                    if m == 0 and k % 2 == 0:
                        if p_ == 0:
                            mm._wait_ge(s_t0, 32)
                        else:
                            mm._wait_ge(
                                s_slot[j], 16 * (p_ // SLOTS + (1 if j else 0))
                            )
                    elif m == 0 and k == 1:
                        mm._wait_ge(s_t1, 32)
                    if m == M_TILES - 1 and k % 2 == 1 and 1 <= p_ <= PAIRS - SLOTS:
                        # pair fully streamed at retire -> slot reusable
                        mm.then_inc(s_mm, 1)
                    if k == K_TILES - 1 and m >= 1:
                        # bank m-1's systolic drain is covered by this MM's
                        # strictly-later end
                        mm.then_inc(s_fin, 1)
            # trailing dummies cover bank 3's drain (MM ends are strictly
            # ordered; two N=1 MMs give >= the ~53ns systolic-drain margin)
            for _ in range(2):
                tensor.matmul(
                    warm_ps[:], warm_sb[0:1, 0:1], warm_sb[0:1, 1:2],
                    start=True, stop=True,
                ).then_inc(s_fin, 1)

    nc.finalize()
    return nc


def _get_program():
    if "v2" not in _PROGRAM_CACHE:
        _PROGRAM_CACHE["v2"] = _build_program()
    return _PROGRAM_CACHE["v2"]


def _patch_walrus_max_sem(cap=64):
    """Append --max-sem-num to the walrus NEFF build. The stock NEFF epilogue
    clears the whole 256-semaphore space one EVENT_SEMAPHORE per sem; this
    program references ~20 sems, so capping the allocator shrinks the clear
    range."""
    try:
        import concourse.bass_utils as bu

        if getattr(bu.get_walrus_args, "_max_sem_patched", False):
            return
        orig = bu.get_walrus_args

        def patched(*a, **k):
            return orig(*a, **k) + [f"--max-sem-num={cap}", "--enable-ldw-opt=true"]

        patched._max_sem_patched = True
        bu.get_walrus_args = patched
    except Exception:
        pass


def _ensure_ntff_hook():
    """Make NTFF profiling under axon work (BASS_TRACE=1): the image's antenv
    package lacks the axon_hooks holder module, so synthesize it and register
    the ctypes-based profile hook from trn_agent_boot. Best-effort."""
    import types

    try:
        import antenv

        try:
            from antenv.axon_hooks import get_axon_ntff_profile_hook  # noqa: F401

            return  # already present and registered
        except ImportError:
            pass
        mod = types.ModuleType("antenv.axon_hooks")
        _holder = [None]
        mod.set_axon_ntff_profile_hook = lambda h: _holder.__setitem__(0, h)
        mod.get_axon_ntff_profile_hook = lambda: _holder[0]
        sys.modules["antenv.axon_hooks"] = mod
        antenv.axon_hooks = mod

        from trn_agent_boot.trn_boot import _ntff_profile_via_ctypes

        hook = _ntff_profile_via_ctypes("/opt/axon/libaxon_pjrt.so")
        mod.set_axon_ntff_profile_hook(hook)
    except Exception:
        pass


def _softmax_weights(logits, flat_idx, seg):
    """Exact fp32 per-set softmax -> dense fp32 weight matrix (8192, 2048)."""
    segmax = np.full(NUM_SETS, -np.inf, dtype=np.float32)
    np.maximum.at(segmax, seg, logits)
    e = np.exp(logits - segmax[seg])
    den = np.zeros(NUM_SETS, dtype=np.float32)
    np.add.at(den, seg, e)
    w = e / den[seg]
    Wf = np.zeros((NUM_GENESETS, NUM_SETS), dtype=np.float32)
    Wf[flat_idx, seg] = w
    return Wf


def kernel(**inputs):
    global LAST_RESULTS
    G = np.asarray(inputs["geneset_features"], dtype=np.float32)
    logits = np.asarray(inputs["attn_logits"], dtype=np.float32)
    flat_idx = np.asarray(inputs["flat_idx"]).astype(np.int64)
    seg = np.asarray(inputs["segment_ids"]).astype(np.int64)

    # Host-side layout prep: softmax weights scattered into the sparse
    # aggregation matrix (member sets are sampled without replacement, so
    # (idx, seg) pairs are unique and the fancy assignment is collision-free).
    Wf = _softmax_weights(logits, flat_idx, seg)

    GbT = np.ascontiguousarray(G.T.astype(ml_dtypes.bfloat16))
    Wb = Wf.astype(ml_dtypes.bfloat16)
    in_maps = []
    for c in range(N_CORES):
        bg, sg = divmod(c, SG)
        gt = GbT[:, bg * B_C : (bg + 1) * B_C].reshape(K_TILES, P, B_C)
        wq = Wb[:, sg * S_C : (sg + 1) * S_C].reshape(K_TILES, P, S_C)
        gw = np.concatenate([gt, wq], axis=2)  # (K_TILES, P, FD) bf16
        # fuse k-tile pairs: (PAIRS, P, 2*FD)
        gw = (
            gw.reshape(PAIRS, 2, P, FD)
            .transpose(0, 2, 1, 3)
            .reshape(PAIRS, P, PFD)
        )
        in_maps.append({"gw": np.ascontiguousarray(gw)})

    from concourse.bass_utils import run_bass_kernel_spmd

    _patch_walrus_max_sem()
    _ensure_ntff_hook()
    nc = _get_program()
    res = run_bass_kernel_spmd(nc, in_maps, core_ids=list(range(N_CORES)))
    LAST_RESULTS = res

    out = np.empty((BATCH, NUM_SETS), dtype=np.float32)
    for c in range(N_CORES):
        bg, sg = divmod(c, SG)
        blk = res.results[c]["out"].astype(np.float32)  # (P, M_TILES*S_C)
        blk = blk.reshape(P, M_TILES, S_C).transpose(1, 0, 2).reshape(B_C, S_C)
        out[bg * B_C : (bg + 1) * B_C, sg * S_C : (sg + 1) * S_C] = blk
    return out


# revision 8
# speedup vs baseline: 1.0214x; 1.0214x over previous
"""Trainium2 Bass kernel for CellPathwayAttentionAggregator (segment-reduce).

Math: out[b, s] = sum_{i in set s} softmax_s(attn_logits)[i] * G[b, flat_idx[i]]

Device decomposition (per core): dense bf16 matmul
    out[b, s] = sum_g G[b, g] * W[g, s]
where W[g, s] = softmax-normalized weight of member (g, s), scattered on the
host as pure layout prep (softmax is exact fp32 host math folded into W).

Sharding: 8 cores = 2 batch groups (512 rows) x 4 set groups (512 sets).
Each core accumulates a (512 x 512) output block over K=8192 in fp32 PSUM
(4 batch-subtile PSUM banks, N=512 moving operand). The PE streams the
moving operand at 1 bf16 column/cycle -> 256 MMs x 512 cyc = 54.6us floor;
the program is built so everything else (DMA 16.7MB @ ~350GB/s, copies,
output DMA) hides under or hangs minimally off that floor:

  - k-tiles DMA'd in PAIRS (512KB, 4KB/partition) alternating the Sync/ACT
    HWDGE rings; 8 pair-slots in SBUF; pair 0 is split into 4 half-tile
    DMAs (2 per ring) with dedicated sems so the first real MM starts ~1us
    after block start.
  - PE warmup MMs (N=1, ~25ns) bridge the gap until tile 0 lands,
    keeping the HAM clock-gate ramping; warm operand is DVE-memset at block
    entry (the 4 framework const-AP gpsimd memsets are removed from the
    entry block -- they cost ~1.9us of all-engine barrier wait).
  - no tensor.drain(): bank m's PSUM->SBUF copy is gated on the *retire*
    of the k=63 MM of bank m+1 (MM ends are strictly ordered, so one-MM
    margin covers the systolic drain); bank 3 is covered by a trailing
    dummy MM. Copies run split DVE/ACT, then one 512KB output DMA with no
    completion wait (the Block-exit queue flush guarantees delivery).
"""

import sys

if "/opt/trn_rl_repo" not in sys.path:
    sys.path.insert(0, "/opt/trn_rl_repo")

import ml_dtypes
import numpy as np

NUM_SETS = 2048
NUM_GENESETS = 8192
BATCH = 1024
N_CORES = 8
BG, SG = 2, 4  # batch groups x set groups (BG*SG == N_CORES)
B_C = BATCH // BG  # 512 batch rows per core
S_C = NUM_SETS // SG  # 512 sets per core
P = 128
K_TILES = NUM_GENESETS // P  # 64
M_TILES = B_C // P  # 4
PAIRS = K_TILES // 2  # 32
SLOTS = 12  # pair slots in SBUF (24 tiles, 6MB)
FD = B_C + S_C  # 1024 bf16 per partition per k-tile
PFD = 2 * FD  # one pair
WARMUPS = 48

_PROGRAM_CACHE = {}
LAST_RESULTS = None  # BassKernelResults of the most recent run (for profiling)


def _strip_const_ap_memsets(nc):
    """Remove the 4 framework const-AP gpsimd memsets from the entry block.
    They run on GpSimd while every other engine sits at the init barrier
    (~1.9us); this program doesn't use any const AP."""
    try:
        import concourse.mybir as mybir

        entry = nc.main_func.blocks[0]
        drop = [
            inst
            for inst in entry.instructions
            if isinstance(inst, mybir.InstMemset)
            and inst.outs
            and "const-" in getattr(inst.outs[0], "memsetref", "")
        ]
        for inst in drop:
            entry.instructions.remove(inst)
    except Exception:
        pass


def _build_program():
    import concourse.mybir as mybir
    from concourse import bacc
    from contextlib import ExitStack

    f32 = mybir.dt.float32
    bf16 = mybir.dt.bfloat16

    nc = bacc.Bacc("TRN2", target_bir_lowering=False, debug=False)
    _strip_const_ap_memsets(nc)
    gw_d = nc.dram_tensor("gw", [PAIRS, P, PFD], bf16, kind="ExternalInput")
    # out rows are (m, p)-major on the device; host transposes for free
    out_d = nc.dram_tensor("out", [P, M_TILES * S_C], bf16, kind="ExternalOutput")

    with ExitStack() as ctx:
        gw_sb = ctx.enter_context(nc.sbuf_tensor([P, SLOTS, PFD], bf16))
        warm_sb = ctx.enter_context(nc.sbuf_tensor([1, 2], bf16))  # never written
        act_sb = ctx.enter_context(nc.sbuf_tensor([1, 2], f32))
        o_sb = ctx.enter_context(nc.sbuf_tensor([P, M_TILES * S_C], bf16))
        acc_ps = ctx.enter_context(nc.psum_tensor([P, M_TILES, S_C], f32))
        warm_ps = ctx.enter_context(nc.psum_tensor([1, 1], f32))
        s_t0 = ctx.enter_context(nc.semaphore(name="s_t0"))
        s_t1 = ctx.enter_context(nc.semaphore(name="s_t1"))
        s_slot = [
            ctx.enter_context(nc.semaphore(name=f"s_slot{j}")) for j in range(SLOTS)
        ]
        s_mm = ctx.enter_context(nc.semaphore(name="s_mm"))
        s_fin = ctx.enter_context(nc.semaphore(name="s_fin"))
        s_o01 = ctx.enter_context(nc.semaphore(name="s_o01"))
        s_o2 = ctx.enter_context(nc.semaphore(name="s_o2"))
        s_done = ctx.enter_context(nc.semaphore(name="s_done"))
        s_warm = ctx.enter_context(nc.semaphore(name="s_warm"))
        # pre-block DMA issue: these run right after the init barrier,
        # ~2us before the Block's own entry barrier completes, so tile 0 is
        # landing while the engines are still syncing into the block
        nc.vector.memset(warm_sb[:], 1.0).then_inc(s_warm, 1)
        nc.sync.dma_start(gw_sb[0:64, 0, 0:FD], gw_d[0, 0:64, 0:FD]).then_inc(
            s_t0, 16
        )
        nc.scalar.dma_start(gw_sb[64:128, 0, 0:FD], gw_d[0, 64:128, 0:FD]).then_inc(
            s_t0, 16
        )
        nc.sync.dma_start(gw_sb[0:64, 0, FD:PFD], gw_d[0, 0:64, FD:PFD]).then_inc(
            s_t1, 16
        )
        nc.scalar.dma_start(
            gw_sb[64:128, 0, FD:PFD], gw_d[0, 64:128, FD:PFD]
        ).then_inc(s_t1, 16)
        nc.scalar.dma_start(gw_sb[:, 1, :], gw_d[1, :, :]).then_inc(s_slot[1], 16)
        nc.sync.dma_start(gw_sb[:, 2, :], gw_d[2, :, :]).then_inc(s_slot[2], 16)

        block = ctx.enter_context(nc.Block(no_gpsimd_drain=True))

        def ring_body(eng, r):
            # pairs 0-2 issued pre-block; even pairs on Sync, odd on ACT
            for p_ in range(4 - r, PAIRS, 2):
                if p_ >= SLOTS:
                    eng.wait_ge(s_mm, p_ - SLOTS + 1)
                eng.dma_start(gw_sb[:, p_ % SLOTS, :], gw_d[p_, :, :]).then_inc(
                    s_slot[p_ % SLOTS], 16
                )

        @block.sync
        def _(sync):
            ring_body(sync, 0)  # pairs 4, 6, ...
            sync.wait_ge(s_o01, 2)
            sync.dma_start(
                out_d[:, 0 : 2 * S_C], o_sb[:, 0 : 2 * S_C]
            ).then_inc(s_done, 16)
            # no completion wait: Block-exit queue flush delivers the bytes

        @block.scalar
        def _(scalar):
            ring_body(scalar, 1)  # pairs 3, 5, ...
            # dummy act: hoists the ~1.3us ACT_TABLE_LOAD into DMA-paced time
            scalar.activation(
                act_sb[0:1, 0:1], act_sb[0:1, 1:2], mybir.ActivationFunctionType.Copy
            )
            scalar.wait_ge(s_fin, 2)
            scalar.activation(
                o_sb[:, S_C : 2 * S_C], acc_ps[:, 1, :],
                mybir.ActivationFunctionType.Copy,
            ).then_inc(s_o01, 1)
            scalar.wait_ge(s_fin, 5)
            scalar.activation(
                o_sb[:, 3 * S_C : 4 * S_C], acc_ps[:, 3, :],
                mybir.ActivationFunctionType.Copy,
            )
            scalar.wait_ge(s_o2, 1)
            scalar.dma_start(
                out_d[:, 2 * S_C : 4 * S_C], o_sb[:, 2 * S_C : 4 * S_C]
            ).then_inc(s_done, 16)

        @block.vector
        def _(vector):
            vector.wait_ge(s_fin, 1)
            vector.tensor_copy(
                o_sb[:, 0:S_C], acc_ps[:, 0, :]
            ).then_inc(s_o01, 1)
            vector.wait_ge(s_fin, 3)
            vector.tensor_copy(
                o_sb[:, 2 * S_C : 3 * S_C], acc_ps[:, 2, :]
            ).then_inc(s_o2, 1)

        @block.gpsimd
        def _(gpsimd):
            # diagnostics only: slice ends reveal DMA-landing times
            gpsimd.wait_ge(s_t0, 32)
            gpsimd.wait_ge(s_t1, 32)
            gpsimd.wait_ge(s_slot[1], 16)
            gpsimd.wait_ge(s_slot[3], 16)
            gpsimd.wait_ge(s_mm, 5)
            gpsimd.wait_ge(s_fin, 5)

        @block.tensor
        def _(tensor):
            # dependency-free warmups keep the HAM clock-gate ramping
            # while tile 0 streams in (warm_sb memset by DVE at block entry)
            tensor.matmul(
                warm_ps[:], warm_sb[0:1, 0:1], warm_sb[0:1, 1:2],
                start=True, stop=True,
            )._wait_ge(s_warm, 1)
            for _ in range(WARMUPS - 1):
                tensor.matmul(
                    warm_ps[:], warm_sb[0:1, 0:1], warm_sb[0:1, 1:2],
                    start=True, stop=True,
                )
            for k in range(K_TILES):
                p_ = k // 2
                j = p_ % SLOTS
                base = (k % 2) * FD
                tile = gw_sb[:, j, base : base + FD]
                for m in range(M_TILES):
                    mm = tensor.matmul(
                        acc_ps[:, m, :],
                        tile[:, m * P : (m + 1) * P],
                        tile[:, B_C:FD],
                        start=(k == 0),
                        stop=(k == K_TILES - 1),
                    )
                    if m == 0 and k % 2 == 0:
                        if p_ == 0:
                            mm._wait_ge(s_t0, 32)
                        else:
                            mm._wait_ge(
                                s_slot[j], 16 * (p_ // SLOTS + (1 if j else 0))
                            )
                    elif m == 0 and k == 1:
                        mm._wait_ge(s_t1, 32)
                    if m == M_TILES - 1 and k % 2 == 1 and 1 <= p_ <= PAIRS - SLOTS:
                        # pair fully streamed at retire -> slot reusable
                        mm.then_inc(s_mm, 1)
                    if k == K_TILES - 1 and m >= 1:
                        # bank m-1's systolic drain is covered by this MM's
                        # strictly-later end
                        mm.then_inc(s_fin, 1)
            # trailing dummies cover bank 3's drain (MM ends are strictly
            # ordered; two N=1 MMs give >= the ~53ns systolic-drain margin)
            for _ in range(2):
                tensor.matmul(
                    warm_ps[:], warm_sb[0:1, 0:1], warm_sb[0:1, 1:2],
                    start=True, stop=True,
                ).then_inc(s_fin, 1)

    nc.finalize()
    return nc


def _get_program():
    if "v2" not in _PROGRAM_CACHE:
        _PROGRAM_CACHE["v2"] = _build_program()
    return _PROGRAM_CACHE["v2"]


def _patch_walrus_max_sem(cap=64):
    """Append --max-sem-num to the walrus NEFF build. The stock NEFF epilogue
    clears the whole 256-semaphore space one EVENT_SEMAPHORE per sem; this
    program references ~20 sems, so capping the allocator shrinks the clear
    range."""
    try:
        import concourse.bass_utils as bu

        if getattr(bu.get_walrus_args, "_max_sem_patched", False):
            return
        orig = bu.get_walrus_args

        def patched(*a, **k):
            return orig(*a, **k) + [f"--max-sem-num={cap}", "--enable-ldw-opt=true"]

        patched._max_sem_patched = True
        bu.get_walrus_args = patched
    except Exception:
        pass


def _ensure_ntff_hook():
    """Make NTFF profiling under axon work (BASS_TRACE=1): the image's antenv
    package lacks the axon_hooks holder module, so synthesize it and register
    the ctypes-based profile hook from trn_agent_boot. Best-effort."""
    import types

    try:
        import antenv

        try:
            from antenv.axon_hooks import get_axon_ntff_profile_hook  # noqa: F401

            return  # already present and registered
        except ImportError:
            pass
        mod = types.ModuleType("antenv.axon_hooks")
        _holder = [None]
        mod.set_axon_ntff_profile_hook = lambda h: _holder.__setitem__(0, h)
        mod.get_axon_ntff_profile_hook = lambda: _holder[0]
        sys.modules["antenv.axon_hooks"] = mod
        antenv.axon_hooks = mod

        from trn_agent_boot.trn_boot import _ntff_profile_via_ctypes

        hook = _ntff_profile_via_ctypes("/opt/axon/libaxon_pjrt.so")
        mod.set_axon_ntff_profile_hook(hook)
    except Exception:
        pass


def _softmax_weights(logits, flat_idx, seg):
    """Exact fp32 per-set softmax -> dense fp32 weight matrix (8192, 2048)."""
    segmax = np.full(NUM_SETS, -np.inf, dtype=np.float32)
    np.maximum.at(segmax, seg, logits)
    e = np.exp(logits - segmax[seg])
    den = np.zeros(NUM_SETS, dtype=np.float32)
    np.add.at(den, seg, e)
    w = e / den[seg]
    Wf = np.zeros((NUM_GENESETS, NUM_SETS), dtype=np.float32)
    Wf[flat_idx, seg] = w
    return Wf


def kernel(**inputs):
    global LAST_RESULTS
    G = np.asarray(inputs["geneset_features"], dtype=np.float32)
    logits = np.asarray(inputs["attn_logits"], dtype=np.float32)
    flat_idx = np.asarray(inputs["flat_idx"]).astype(np.int64)
    seg = np.asarray(inputs["segment_ids"]).astype(np.int64)

    # Host-side layout prep: softmax weights scattered into the sparse
    # aggregation matrix (member sets are sampled without replacement, so
    # (idx, seg) pairs are unique and the fancy assignment is collision-free).
    Wf = _softmax_weights(logits, flat_idx, seg)

    GbT = np.ascontiguousarray(G.T.astype(ml_dtypes.bfloat16))
    Wb = Wf.astype(ml_dtypes.bfloat16)
    in_maps = []
    for c in range(N_CORES):
        bg, sg = divmod(c, SG)
        gt = GbT[:, bg * B_C : (bg + 1) * B_C].reshape(K_TILES, P, B_C)
        wq = Wb[:, sg * S_C : (sg + 1) * S_C].reshape(K_TILES, P, S_C)
        gw = np.concatenate([gt, wq], axis=2)  # (K_TILES, P, FD) bf16
        # fuse k-tile pairs: (PAIRS, P, 2*FD)
        gw = (
            gw.reshape(PAIRS, 2, P, FD)
            .transpose(0, 2, 1, 3)
            .reshape(PAIRS, P, PFD)
        )
        in_maps.append({"gw": np.ascontiguousarray(gw)})

    from concourse.bass_utils import run_bass_kernel_spmd

    _patch_walrus_max_sem()
    _ensure_ntff_hook()
    nc = _get_program()
    res = run_bass_kernel_spmd(nc, in_maps, core_ids=list(range(N_CORES)))
    LAST_RESULTS = res

    out = np.empty((BATCH, NUM_SETS), dtype=np.float32)
    for c in range(N_CORES):
        bg, sg = divmod(c, SG)
        blk = res.results[c]["out"].astype(np.float32)  # (P, M_TILES*S_C)
        blk = blk.reshape(P, M_TILES, S_C).transpose(1, 0, 2).reshape(B_C, S_C)
        out[bg * B_C : (bg + 1) * B_C, sg * S_C : (sg + 1) * S_C] = blk
    return out
